# revision 21
# baseline (speedup 1.0000x reference)
"""NetVLAD Trainium2 kernel (8-core SPMD, data-parallel over batch).

Per-core pipeline, 8 samples processed as 4 PAIRS (A=2p, B=2p+1) so every
engine runs full 128-partition width:
  mm1:  s^T[128(kA|kB), hw] — W stationary, col-group tiled per sample
  ACT:  e^T = exp(s^T + b2) (one op per pair)
  PE :  row-group tiled transposes -> e natural [128, c, (kA|kB)]
  DVE:  softmax sums + one broadcast mult -> a
  mm2:  v^T [128(kA|kB), d] col-group tiled; a_sum rows via ones-stationary;
        diag(a_sum) @ [C^T;C^T] folded into the same PSUM accumulation
  norms: exact intra + global L2; single batched sqrt at the tail
  out:  v_hat^T [(2*64), 512] f32 per pair; host fixes layout
"""

import os
import sys

import numpy as np

for _p in ("/opt/trn_rl_repo", "/root/.axon_site/_ro/trn_rl_repo"):
    if os.path.isdir(_p) and _p not in sys.path:
        sys.path.append(_p)

from contextlib import ExitStack

from concourse import bacc, bass, mybir, tile
from concourse.bass_utils import run_bass_kernel_spmd

N_CORES = 8
NSAMP = 8        # samples per core
NPAIR = NSAMP // 2
HW = 1024        # H*W
D = 512
K = 64
DC = D // 128    # d-chunks of 128
HC = HW // 128   # hw-chunks of 128
F16 = mybir.dt.float16
F32 = mybir.dt.float32
EPS = 1e-12
MULT = mybir.AluOpType.mult
ADD = mybir.AluOpType.add

LAST_EXEC_NS = None
LAST_RESULTS = None

_CACHE = {}


def _build_program():
    nc = bacc.Bacc("TRN2", target_bir_lowering=False, debug=False)

    xT_d = nc.dram_tensor("xT", [NSAMP, 128, DC * HW], F16, kind="ExternalInput").ap()
    xn_d = nc.dram_tensor("xn", [NPAIR, 128, 2 * HC * D], F16, kind="ExternalInput").ap()
    W_d = nc.dram_tensor("Wt", [128, DC * K], F16, kind="ExternalInput").ap()
    b2_d = nc.dram_tensor("bcol2", [128, 1], F32, kind="ExternalInput").ap()
    CT2_d = nc.dram_tensor("CT2", [128, D], F16, kind="ExternalInput").ap()
    eye128_d = nc.dram_tensor("eye128", [128, 128], F16, kind="ExternalInput").ap()
    ones128_d = nc.dram_tensor("ones128", [128, 1], F16, kind="ExternalInput").ap()
    ones2c_d = nc.dram_tensor("ones2c", [128, 2], F32, kind="ExternalInput").ap()
    sel2_d = nc.dram_tensor("sel2", [2, 128], F32, kind="ExternalInput").ap()
    one1_d = nc.dram_tensor("one1", [1, 1], F16, kind="ExternalInput").ap()
    out_d = nc.dram_tensor("out", [NSAMP * K, D], F32, kind="ExternalOutput").ap()

    with tile.TileContext(nc) as tc, ExitStack() as ctx:
        const = ctx.enter_context(tc.tile_pool(name="const", bufs=1))
        xT_pool = ctx.enter_context(tc.tile_pool(name="xTp", bufs=3))
        xn_pool = ctx.enter_context(tc.tile_pool(name="xnp", bufs=2))
        eT_pool = ctx.enter_context(tc.tile_pool(name="eTp", bufs=2))
        a_pool = ctx.enter_context(tc.tile_pool(name="ap", bufs=2))
        sm_pool = ctx.enter_context(tc.tile_pool(name="smp", bufs=2))
        scr_pool = ctx.enter_context(tc.tile_pool(name="scrp", bufs=2))
        vraw_pool = ctx.enter_context(tc.tile_pool(name="vrawp", bufs=1))
        vo_pool = ctx.enter_context(tc.tile_pool(name="vop", bufs=2))

        ps_sT = ctx.enter_context(tc.tile_pool(name="ps_sT", bufs=2, space="PSUM"))
        ps_e = ctx.enter_context(tc.tile_pool(name="ps_e", bufs=2, space="PSUM"))
        ps_v = ctx.enter_context(tc.tile_pool(name="ps_v", bufs=2, space="PSUM"))
        ps_t = ctx.enter_context(tc.tile_pool(name="ps_t", bufs=2, space="PSUM"))

        # constants
        W_sb = const.tile([128, DC, K], F16)
        nc.gpsimd.dma_start(W_sb[:].rearrange("p dc k -> p (dc k)"), W_d[:])
        b2_sb = const.tile([128, 1], F32)
        nc.gpsimd.dma_start(b2_sb[:], b2_d[:])
        CT2_sb = const.tile([128, D], F16)
        nc.gpsimd.dma_start(CT2_sb[:], CT2_d[:])
        eye128_sb = const.tile([128, 128], F16)
        nc.gpsimd.dma_start(eye128_sb[:], eye128_d[:])
        ones128_sb = const.tile([128, 1], F16)
        nc.gpsimd.dma_start(ones128_sb[:], ones128_d[:])
        ones2c_sb = const.tile([128, 2], F32)
        nc.gpsimd.dma_start(ones2c_sb[:], ones2c_d[:])
        sel2_sb = const.tile([2, 128], F32)
        nc.gpsimd.dma_start(sel2_sb[:], sel2_d[:])
        one1_sb = const.tile([1, 1], F16)
        nc.gpsimd.dma_start(one1_sb[:], one1_d[:])
        prod_all = const.tile([128, NPAIR], F32)

        vraws = []
        state = {}

        xts = {}

        def load_xT(p):
            nA, nB = 2 * p, 2 * p + 1
            xTA = xT_pool.tile([128, DC, HW], F16, name=f"xTA_{p}", tag="xTA")
            if p == 0:
                half = DC * HW // 2
                nc.sync.dma_start(
                    xTA[:, 0:DC // 2, :].rearrange("p dc f -> p (dc f)"),
                    xT_d[nA, :, 0:half],
                )
                nc.sync.dma_start(
                    xTA[:, DC // 2:DC, :].rearrange("p dc f -> p (dc f)"),
                    xT_d[nA, :, half:],
                )
            else:
                nc.sync.dma_start(xTA[:].rearrange("p dc f -> p (dc f)"), xT_d[nA])
            xTB = xT_pool.tile([128, DC, HW], F16, name=f"xTB_{p}", tag="xTB")
            nc.sync.dma_start(xTB[:].rearrange("p dc f -> p (dc f)"), xT_d[nB])
            xts[p] = (xTA, xTB)

        def phase1(p):
            nA, nB = 2 * p, 2 * p + 1
            xTA, xTB = xts.pop(p)
            if p + 1 < NPAIR:
                load_xT(p + 1)
            xn_t = xn_pool.tile([128, 2, HC, D], F16, name=f"xn_{p}", tag="xn")
            q = HC * D // 2
            for iq in range(4):
                nc.sync.dma_start(
                    xn_t[:, iq // 2, (iq % 2) * (HC // 2):(iq % 2 + 1) * (HC // 2), :]
                    .rearrange("p c d -> p (c d)"),
                    xn_d[p, :, iq * q:(iq + 1) * q],
                )
            xnA = xn_t[:, 0]
            xnB = xn_t[:, 1]

            # ---- mm1: s^T halves, col-group tiled, dc-outer; exp per half ----
            eT_sb = eT_pool.tile([128, HW], F16, name=f"eT_{p}", tag="eT")
            sT_h = [
                ps_sT.tile([128, 512], F32, name=f"sT_{p}_{h}", tag="sT")
                for h in range(2)
            ]
            for dc in range(DC):
                for h in range(2):
                    hs = slice(h * 512, (h + 1) * 512)
                    nc.tensor.matmul(
                        sT_h[h][0:K, :], lhsT=W_sb[:, dc, :],
                        rhs=xTA[:, dc, hs],
                        start=(dc == 0), stop=(dc == DC - 1),
                        skip_group_check=True,
                    )
                    nc.tensor.matmul(
                        sT_h[h][K:128, :], lhsT=W_sb[:, dc, :],
                        rhs=xTB[:, dc, hs],
                        start=(dc == 0), stop=(dc == DC - 1),
                        skip_group_check=True,
                    )
            for h in range(2):
                hs = slice(h * 512, (h + 1) * 512)
                nc.scalar.activation(
                    eT_sb[:, hs], sT_h[h][:],
                    mybir.ActivationFunctionType.Exp,
                    bias=b2_sb[:], scale=1.0,
                )

            # ---- transposes + halved softmax pipeline ----
            e_ps = ps_e.tile([128, HC, 128], F16, name=f"e_{p}", tag="e")
            a_sb = a_pool.tile([128, HC, 128], F16, name=f"a_{p}", tag="a")
            HH = HC // 2
            for g in range(2):
                for c in range(g * HH, (g + 1) * HH):
                    cs = slice(c * 128, (c + 1) * 128)
                    nc.tensor.transpose(
                        e_ps[:, c, :], eT_sb[:, cs], eye128_sb[:]
                    )
                gsl = slice(g * HH, (g + 1) * HH)
                e_v = e_ps[:, gsl, :].rearrange("p c (s k) -> p (c s) k", s=2)
                S_sb = sm_pool.tile([128, 2 * HH], F32, name=f"S_{p}_{g}", tag="S")
                nc.vector.reduce_sum(S_sb[:], e_v, axis=mybir.AxisListType.X)
                r_sb = sm_pool.tile([128, 2 * HH], F32, name=f"r_{p}_{g}", tag="r")
                nc.vector.reciprocal(r_sb[:], S_sb[:])
                nc.vector.tensor_tensor(
                    out=a_sb[:, gsl, :].rearrange("p c (s k) -> p (c s) k", s=2),
                    in0=e_v,
                    in1=r_sb[:].unsqueeze(-1).broadcast_to((128, 2 * HH, K)),
                    op=MULT,
                )

            state[p] = (a_sb, xnA, xnB, eT_sb)

        def phase2(p):
            a_sb, xnA, xnB, eT_sb = state.pop(p)
            # ---- mm2 col-group tiled + a_sum rows + paired diag matmul ----
            v_ps = ps_v.tile([128, D], F32, name=f"v_{p}", tag="v")
            tiny_ps = ps_t.tile([128, 144], F32, name=f"tiny_{p}", tag="tiny")
            for c in range(HC):
                nc.tensor.matmul(
                    tiny_ps[0:1, 0:128], lhsT=ones128_sb[:], rhs=a_sb[:, c, :],
                    start=(c == 0), stop=(c == HC - 1), skip_group_check=True,
                )
            for c in range(HC):
                nc.tensor.matmul(
                    v_ps[0:K, :], lhsT=a_sb[:, c, 0:K], rhs=xnA[:, c, :],
                    start=(c == 0), stop=False, skip_group_check=True,
                )
                nc.tensor.matmul(
                    v_ps[K:128, :], lhsT=a_sb[:, c, K:128], rhs=xnB[:, c, :],
                    start=(c == 0), stop=False, skip_group_check=True,
                )
            arow_sb = sm_pool.tile([1, 128], F16, name=f"arow_{p}", tag="arow")
            nc.vector.tensor_copy(arow_sb[:], tiny_ps[0:1, 0:128])
            nc.tensor.matmul(
                tiny_ps[:, 128:129], lhsT=arow_sb[:], rhs=one1_sb[:],
                start=True, stop=True, skip_group_check=True,
            )
            diag_sb = sm_pool.tile([128, 128], F16, name=f"diag_{p}", tag="diag")
            nc.vector.tensor_scalar(
                diag_sb[:], eye128_sb[:], tiny_ps[:, 128:129], None, op0=MULT,
            )
            nc.tensor.matmul(
                v_ps[:], lhsT=diag_sb[:], rhs=CT2_sb[:],
                start=False, stop=True, skip_group_check=True,
            )

            # ---- intra-norm pieces (sqrt deferred to batched tail) ----
            sq_sb = scr_pool.tile([128, D], F16, name=f"sq_{p}", tag="sq")
            ssq_sb = sm_pool.tile([128, 1], F32, name=f"ssq_{p}", tag="ssq")
            nc.scalar.activation(
                sq_sb[:], v_ps[:],
                mybir.ActivationFunctionType.Square,
                accum_out=ssq_sb[:],
            )
            vraw_sb = vraw_pool.tile([128, D], F32, name=f"vraw_{p}", tag=f"vraw{p}")
            nc.scalar.copy(vraw_sb[:], v_ps[:])
            vraws.append(vraw_sb)

            s1_sb = sm_pool.tile([128, 1], F32, name=f"s1_{p}", tag="s1")
            nc.vector.tensor_scalar(s1_sb[:], ssq_sb[:], EPS, None, op0=ADD)
            rec_sb = sm_pool.tile([128, 1], F32, name=f"rec_{p}", tag="rec")
            nc.vector.reciprocal(rec_sb[:], s1_sb[:])
            t_sb = sm_pool.tile([128, 1], F32, name=f"t_{p}", tag="t")
            nc.vector.tensor_mul(t_sb[:], ssq_sb[:], rec_sb[:])
            # per-sample block sums: [2,1] = ones2c^T @ t
            nc.tensor.matmul(
                tiny_ps[0:2, 129:130], lhsT=ones2c_sb[:], rhs=t_sb[:],
                start=True, stop=True, skip_group_check=True,
            )
            tote_sb = sm_pool.tile([2, 1], F32, name=f"tote_{p}", tag="tote")
            nc.vector.tensor_scalar(tote_sb[:], tiny_ps[0:2, 129:130], EPS, None,
                                    op0=ADD)
            # broadcast per-sample totals back to the 128 rows
            nc.tensor.matmul(
                tiny_ps[:, 130:131], lhsT=sel2_sb[:], rhs=tote_sb[:],
                start=True, stop=True, skip_group_check=True,
            )
            nc.vector.tensor_tensor(
                out=prod_all[:, p:p + 1], in0=s1_sb[:], in1=tiny_ps[:, 130:131],
                op=MULT,
            )

        # HAM warmup: dummy matmuls on memset data while first DMAs land
        wu_sb = const.tile([128, 144], F16)
        nc.gpsimd.memset(wu_sb[:], 1.0)
        wu_ps = ps_t.tile([128, 144], F32, name="wu_ps", tag="tiny")
        for _ in range(48):
            nc.tensor.matmul(
                wu_ps[:, 0:144], lhsT=wu_sb[:, 0:128], rhs=wu_sb[:, 0:144],
                start=True, stop=True, skip_group_check=True,
            )
        load_xT(0)
        for p in range(NPAIR + 1):
            if p < NPAIR:
                phase1(p)
            if p >= 1:
                phase2(p - 1)

        # ---- tail ----
        sqall_sb = const.tile([128, NPAIR], F32)
        alpha_sb = const.tile([128, NPAIR], F32)

        def finalize(plo, phi):
            nc.scalar.activation(
                sqall_sb[:, plo:phi], prod_all[:, plo:phi],
                mybir.ActivationFunctionType.Sqrt,
            )
            nc.vector.reciprocal(alpha_sb[:, plo:phi], sqall_sb[:, plo:phi])
            for p in range(plo, phi):
                vo_sb = vo_pool.tile([128, D], F32, name=f"vo_{p}", tag="vo")
                if p % 2 == 0:
                    nc.scalar.activation(
                        vo_sb[:], vraws[p][:],
                        mybir.ActivationFunctionType.Copy,
                        scale=alpha_sb[:, p:p + 1],
                    )
                else:
                    nc.vector.tensor_scalar(
                        vo_sb[:], vraws[p][:], alpha_sb[:, p:p + 1], None,
                        op0=MULT,
                    )
                nc.sync.dma_start(out_d[p * 128:(p + 1) * 128, :], vo_sb[:])

        finalize(0, NPAIR - 1)
        finalize(NPAIR - 1, NPAIR)

    nc.compile()
    return nc


def _get_program():
    if "nc" not in _CACHE:
        _CACHE["nc"] = _build_program()
    return _CACHE["nc"]


def _make_consts():
    eye = np.eye(K, dtype=np.float16)
    ones2c = np.zeros((128, 2), dtype=np.float32)
    ones2c[0:K, 0] = 1.0
    ones2c[K:128, 1] = 1.0
    sel2 = np.ascontiguousarray(ones2c.T)
    return {
        "eye128": np.eye(128, dtype=np.float16),
        "ones128": np.ones((128, 1), dtype=np.float16),
        "ones2c": ones2c,
        "sel2": sel2,
        "one1": np.ones((1, 1), dtype=np.float16),
    }


def kernel(x, W_assign, b_assign, C):
    global LAST_EXEC_NS, LAST_RESULTS

    x = np.asarray(x, dtype=np.float32).reshape(64, HW, D)
    W_assign = np.asarray(W_assign, dtype=np.float32)
    b_assign = np.asarray(b_assign, dtype=np.float32)
    C = np.asarray(C, dtype=np.float32)

    W16 = np.ascontiguousarray(
        W_assign.astype(np.float16).reshape(DC, 128, K).transpose(1, 0, 2)
    ).reshape(128, DC * K)
    bcol2 = np.concatenate([b_assign, b_assign]).reshape(128, 1).astype(np.float32)
    CT16 = np.ascontiguousarray(C.T).astype(np.float16)
    CT2 = np.vstack([CT16, CT16])
    consts = _make_consts()

    in_maps = []
    for c in range(N_CORES):
        xs = x[c * NSAMP:(c + 1) * NSAMP]
        xn16 = np.ascontiguousarray(
            xs.reshape(NPAIR, 2, HC, 128, D).transpose(0, 3, 1, 2, 4)
        ).reshape(NPAIR, 128, 2 * HC * D).astype(np.float16)
        xT = xs.transpose(0, 2, 1).reshape(NSAMP, DC, 128, HW)
        xT16 = np.ascontiguousarray(
            xT.transpose(0, 2, 1, 3)
        ).reshape(NSAMP, 128, DC * HW).astype(np.float16)
        in_maps.append({
            "xT": xT16, "xn": xn16, "Wt": W16, "bcol2": bcol2, "CT2": CT2,
            **consts,
        })

    nc = _get_program()
    trace = bool(int(os.environ.get("KERNEL_TRACE", "0")))
    res = run_bass_kernel_spmd(
        nc, in_maps, core_ids=list(range(N_CORES)), trace=trace,
    )
    LAST_RESULTS = res
    LAST_EXEC_NS = res.exec_time_ns

    out = np.empty((64, D * K), dtype=np.float32)
    for c in range(N_CORES):
        vT = res.results[c]["out"].reshape(NSAMP, K, D)
        out[c * NSAMP:(c + 1) * NSAMP] = (
            vT.transpose(0, 2, 1).reshape(NSAMP, D * K)
        )
    return out


# revision 22
# speedup vs baseline: 1.1793x; 1.1793x over previous
"""NetVLAD Trainium2 kernel (8-core SPMD, data-parallel over batch).

Per-core pipeline, 8 samples processed as 4 PAIRS (A=2p, B=2p+1) so every
engine runs full 128-partition width:
  mm1:  s^T[128(kA|kB), hw] — W stationary, col-group tiled per sample
  ACT:  e^T = exp(s^T + b2) (one op per pair)
  PE :  row-group tiled transposes -> e natural [128, c, (kA|kB)]
  DVE:  softmax sums + one broadcast mult -> a
  mm2:  v^T [128(kA|kB), d] col-group tiled; a_sum rows via ones-stationary;
        diag(a_sum) @ [C^T;C^T] folded into the same PSUM accumulation
  norms: exact intra + global L2; single batched sqrt at the tail
  out:  v_hat^T [(2*64), 512] f32 per pair; host fixes layout
"""

import os
import sys

import numpy as np

for _p in ("/opt/trn_rl_repo", "/root/.axon_site/_ro/trn_rl_repo"):
    if os.path.isdir(_p) and _p not in sys.path:
        sys.path.append(_p)

from contextlib import ExitStack

from concourse import bacc, bass, mybir, tile
from concourse.bass_utils import run_bass_kernel_spmd

N_CORES = 8
NSAMP = 8        # samples per core
NPAIR = NSAMP // 2
HW = 1024        # H*W
D = 512
K = 64
DC = D // 128    # d-chunks of 128
HC = HW // 128   # hw-chunks of 128
F16 = mybir.dt.float16
F32 = mybir.dt.float32
EPS = 1e-12
MULT = mybir.AluOpType.mult
ADD = mybir.AluOpType.add

LAST_EXEC_NS = None
LAST_RESULTS = None

_CACHE = {}


def _build_program():
    nc = bacc.Bacc("TRN2", target_bir_lowering=False, debug=False)

    xT_d = nc.dram_tensor("xT", [NSAMP, 128, DC * HW], F16, kind="ExternalInput").ap()
    xn_d = nc.dram_tensor("xn", [NPAIR, 128, 2 * HC * D], F16, kind="ExternalInput").ap()
    W_d = nc.dram_tensor("Wt", [128, DC * K], F16, kind="ExternalInput").ap()
    b2_d = nc.dram_tensor("bcol2", [128, 1], F32, kind="ExternalInput").ap()
    CT2_d = nc.dram_tensor("CT2", [128, D], F16, kind="ExternalInput").ap()
    eye128_d = nc.dram_tensor("eye128", [128, 128], F16, kind="ExternalInput").ap()
    ones128_d = nc.dram_tensor("ones128", [128, 1], F16, kind="ExternalInput").ap()
    ones2c_d = nc.dram_tensor("ones2c", [128, 2], F32, kind="ExternalInput").ap()
    sel2_d = nc.dram_tensor("sel2", [2, 128], F32, kind="ExternalInput").ap()
    one1_d = nc.dram_tensor("one1", [1, 1], F16, kind="ExternalInput").ap()
    out_d = nc.dram_tensor("out", [NSAMP * K, D], F32, kind="ExternalOutput").ap()

    with tile.TileContext(nc) as tc, ExitStack() as ctx:
        const = ctx.enter_context(tc.tile_pool(name="const", bufs=1))
        xT_pool = ctx.enter_context(tc.tile_pool(name="xTp", bufs=3))
        xn_pool = ctx.enter_context(tc.tile_pool(name="xnp", bufs=2))
        eT_pool = ctx.enter_context(tc.tile_pool(name="eTp", bufs=3))
        a_pool = ctx.enter_context(tc.tile_pool(name="ap", bufs=3))
        sm_pool = ctx.enter_context(tc.tile_pool(name="smp", bufs=3))
        scr_pool = ctx.enter_context(tc.tile_pool(name="scrp", bufs=2))
        vraw_pool = ctx.enter_context(tc.tile_pool(name="vrawp", bufs=1))
        vo_pool = ctx.enter_context(tc.tile_pool(name="vop", bufs=2))

        ps_sT = ctx.enter_context(tc.tile_pool(name="ps_sT", bufs=2, space="PSUM"))
        ps_e = ctx.enter_context(tc.tile_pool(name="ps_e", bufs=2, space="PSUM"))
        ps_v = ctx.enter_context(tc.tile_pool(name="ps_v", bufs=2, space="PSUM"))
        ps_t = ctx.enter_context(tc.tile_pool(name="ps_t", bufs=2, space="PSUM"))

        # constants
        W_sb = const.tile([128, DC, K], F16)
        nc.gpsimd.dma_start(W_sb[:].rearrange("p dc k -> p (dc k)"), W_d[:])
        b2_sb = const.tile([128, 1], F32)
        nc.gpsimd.dma_start(b2_sb[:], b2_d[:])
        CT2_sb = const.tile([128, D], F16)
        nc.gpsimd.dma_start(CT2_sb[:], CT2_d[:])
        eye128_sb = const.tile([128, 128], F16)
        nc.gpsimd.dma_start(eye128_sb[:], eye128_d[:])
        ones128_sb = const.tile([128, 1], F16)
        nc.gpsimd.dma_start(ones128_sb[:], ones128_d[:])
        ones2c_sb = const.tile([128, 2], F32)
        nc.gpsimd.dma_start(ones2c_sb[:], ones2c_d[:])
        sel2_sb = const.tile([2, 128], F32)
        nc.gpsimd.dma_start(sel2_sb[:], sel2_d[:])
        one1_sb = const.tile([1, 1], F16)
        nc.gpsimd.dma_start(one1_sb[:], one1_d[:])
        prod_all = const.tile([128, NPAIR], F32)

        vraws = []
        state = {}

        xts = {}

        def load_xT(p):
            nA, nB = 2 * p, 2 * p + 1
            xTA = xT_pool.tile([128, DC, HW], F16, name=f"xTA_{p}", tag="xTA")
            if p == 0:
                half = DC * HW // 2
                nc.sync.dma_start(
                    xTA[:, 0:DC // 2, :].rearrange("p dc f -> p (dc f)"),
                    xT_d[nA, :, 0:half],
                )
                nc.sync.dma_start(
                    xTA[:, DC // 2:DC, :].rearrange("p dc f -> p (dc f)"),
                    xT_d[nA, :, half:],
                )
            else:
                nc.sync.dma_start(xTA[:].rearrange("p dc f -> p (dc f)"), xT_d[nA])
            xTB = xT_pool.tile([128, DC, HW], F16, name=f"xTB_{p}", tag="xTB")
            nc.sync.dma_start(xTB[:].rearrange("p dc f -> p (dc f)"), xT_d[nB])
            xts[p] = (xTA, xTB)

        def phase1(p):
            nA, nB = 2 * p, 2 * p + 1
            xTA, xTB = xts.pop(p)
            if p + 1 < NPAIR:
                load_xT(p + 1)
            xn_t = xn_pool.tile([128, 2, HC, D], F16, name=f"xn_{p}", tag="xn")
            q = HC * D // 2
            for iq in range(4):
                nc.sync.dma_start(
                    xn_t[:, iq // 2, (iq % 2) * (HC // 2):(iq % 2 + 1) * (HC // 2), :]
                    .rearrange("p c d -> p (c d)"),
                    xn_d[p, :, iq * q:(iq + 1) * q],
                )
            xnA = xn_t[:, 0]
            xnB = xn_t[:, 1]

            # ---- mm1: s^T halves, col-group tiled, dc-outer; exp per half ----
            eT_sb = eT_pool.tile([128, HW], F16, name=f"eT_{p}", tag="eT")
            sT_h = [
                ps_sT.tile([128, 512], F32, name=f"sT_{p}_{h}", tag="sT")
                for h in range(2)
            ]
            for dc in range(DC):
                for h in range(2):
                    hs = slice(h * 512, (h + 1) * 512)
                    nc.tensor.matmul(
                        sT_h[h][0:K, :], lhsT=W_sb[:, dc, :],
                        rhs=xTA[:, dc, hs],
                        start=(dc == 0), stop=(dc == DC - 1),
                        skip_group_check=True,
                    )
                    nc.tensor.matmul(
                        sT_h[h][K:128, :], lhsT=W_sb[:, dc, :],
                        rhs=xTB[:, dc, hs],
                        start=(dc == 0), stop=(dc == DC - 1),
                        skip_group_check=True,
                    )
            for h in range(2):
                hs = slice(h * 512, (h + 1) * 512)
                nc.scalar.activation(
                    eT_sb[:, hs], sT_h[h][:],
                    mybir.ActivationFunctionType.Exp,
                    bias=b2_sb[:], scale=1.0,
                )

            # ---- transposes + halved softmax pipeline ----
            e_ps = ps_e.tile([128, HC, 128], F16, name=f"e_{p}", tag="e")
            a_sb = a_pool.tile([128, HC, 128], F16, name=f"a_{p}", tag="a")
            HH = HC // 2
            for g in range(2):
                for c in range(g * HH, (g + 1) * HH):
                    cs = slice(c * 128, (c + 1) * 128)
                    nc.tensor.transpose(
                        e_ps[:, c, :], eT_sb[:, cs], eye128_sb[:]
                    )
                gsl = slice(g * HH, (g + 1) * HH)
                e_v = e_ps[:, gsl, :].rearrange("p c (s k) -> p (c s) k", s=2)
                S_sb = sm_pool.tile([128, 2 * HH], F32, name=f"S_{p}_{g}", tag="S")
                nc.vector.reduce_sum(S_sb[:], e_v, axis=mybir.AxisListType.X)
                r_sb = sm_pool.tile([128, 2 * HH], F32, name=f"r_{p}_{g}", tag="r")
                nc.vector.reciprocal(r_sb[:], S_sb[:])
                nc.vector.tensor_tensor(
                    out=a_sb[:, gsl, :].rearrange("p c (s k) -> p (c s) k", s=2),
                    in0=e_v,
                    in1=r_sb[:].unsqueeze(-1).broadcast_to((128, 2 * HH, K)),
                    op=MULT,
                )

            state[p] = (a_sb, xnA, xnB, eT_sb)

        def phase2(p):
            a_sb, xnA, xnB, eT_sb = state.pop(p)
            # ---- mm2 col-group tiled + a_sum rows + paired diag matmul ----
            v_ps = ps_v.tile([128, D], F32, name=f"v_{p}", tag="v")
            tiny_ps = ps_t.tile([128, 144], F32, name=f"tiny_{p}", tag="tiny")
            for c in range(HC):
                nc.tensor.matmul(
                    tiny_ps[0:1, 0:128], lhsT=ones128_sb[:], rhs=a_sb[:, c, :],
                    start=(c == 0), stop=(c == HC - 1), skip_group_check=True,
                )
            for c in range(HC):
                nc.tensor.matmul(
                    v_ps[0:K, :], lhsT=a_sb[:, c, 0:K], rhs=xnA[:, c, :],
                    start=(c == 0), stop=False, skip_group_check=True,
                )
                nc.tensor.matmul(
                    v_ps[K:128, :], lhsT=a_sb[:, c, K:128], rhs=xnB[:, c, :],
                    start=(c == 0), stop=False, skip_group_check=True,
                )
            arow_sb = sm_pool.tile([1, 128], F16, name=f"arow_{p}", tag="arow")
            nc.vector.tensor_copy(arow_sb[:], tiny_ps[0:1, 0:128])
            nc.tensor.matmul(
                tiny_ps[:, 128:129], lhsT=arow_sb[:], rhs=one1_sb[:],
                start=True, stop=True, skip_group_check=True,
            )
            diag_sb = sm_pool.tile([128, 128], F16, name=f"diag_{p}", tag="diag")
            nc.vector.tensor_scalar(
                diag_sb[:], eye128_sb[:], tiny_ps[:, 128:129], None, op0=MULT,
            )
            nc.tensor.matmul(
                v_ps[:], lhsT=diag_sb[:], rhs=CT2_sb[:],
                start=False, stop=True, skip_group_check=True,
            )

            # ---- intra-norm pieces (sqrt deferred to batched tail) ----
            sq_sb = scr_pool.tile([128, D], F16, name=f"sq_{p}", tag="sq")
            ssq_sb = sm_pool.tile([128, 1], F32, name=f"ssq_{p}", tag="ssq")
            nc.scalar.activation(
                sq_sb[:], v_ps[:],
                mybir.ActivationFunctionType.Square,
                accum_out=ssq_sb[:],
            )
            vraw_sb = vraw_pool.tile([128, D], F32, name=f"vraw_{p}", tag=f"vraw{p}")
            nc.scalar.copy(vraw_sb[:], v_ps[:])
            vraws.append(vraw_sb)

            s1_sb = sm_pool.tile([128, 1], F32, name=f"s1_{p}", tag="s1")
            nc.vector.tensor_scalar(s1_sb[:], ssq_sb[:], EPS, None, op0=ADD)
            rec_sb = sm_pool.tile([128, 1], F32, name=f"rec_{p}", tag="rec")
            nc.vector.reciprocal(rec_sb[:], s1_sb[:])
            t_sb = sm_pool.tile([128, 1], F32, name=f"t_{p}", tag="t")
            nc.vector.tensor_mul(t_sb[:], ssq_sb[:], rec_sb[:])
            # per-sample block sums: [2,1] = ones2c^T @ t
            nc.tensor.matmul(
                tiny_ps[0:2, 129:130], lhsT=ones2c_sb[:], rhs=t_sb[:],
                start=True, stop=True, skip_group_check=True,
            )
            tote_sb = sm_pool.tile([2, 1], F32, name=f"tote_{p}", tag="tote")
            nc.vector.tensor_scalar(tote_sb[:], tiny_ps[0:2, 129:130], EPS, None,
                                    op0=ADD)
            # broadcast per-sample totals back to the 128 rows
            nc.tensor.matmul(
                tiny_ps[:, 130:131], lhsT=sel2_sb[:], rhs=tote_sb[:],
                start=True, stop=True, skip_group_check=True,
            )
            nc.vector.tensor_tensor(
                out=prod_all[:, p:p + 1], in0=s1_sb[:], in1=tiny_ps[:, 130:131],
                op=MULT,
            )

        # HAM warmup: dummy matmuls on memset data while first DMAs land
        wu_sb = const.tile([128, 144], F16)
        nc.gpsimd.memset(wu_sb[:], 1.0)
        wu_ps = ps_t.tile([128, 144], F32, name="wu_ps", tag="tiny")
        for _ in range(48):
            nc.tensor.matmul(
                wu_ps[:, 0:144], lhsT=wu_sb[:, 0:128], rhs=wu_sb[:, 0:144],
                start=True, stop=True, skip_group_check=True,
            )
        load_xT(0)
        for p in range(NPAIR + 1):
            if p < NPAIR:
                phase1(p)
            if p >= 1:
                phase2(p - 1)

        # ---- tail ----
        sqall_sb = const.tile([128, NPAIR], F32)
        alpha_sb = const.tile([128, NPAIR], F32)

        def finalize(plo, phi):
            nc.scalar.activation(
                sqall_sb[:, plo:phi], prod_all[:, plo:phi],
                mybir.ActivationFunctionType.Sqrt,
            )
            nc.vector.reciprocal(alpha_sb[:, plo:phi], sqall_sb[:, plo:phi])
            for p in range(plo, phi):
                vo_sb = vo_pool.tile([128, D], F32, name=f"vo_{p}", tag="vo")
                if p % 2 == 0:
                    nc.scalar.activation(
                        vo_sb[:], vraws[p][:],
                        mybir.ActivationFunctionType.Copy,
                        scale=alpha_sb[:, p:p + 1],
                    )
                else:
                    nc.vector.tensor_scalar(
                        vo_sb[:], vraws[p][:], alpha_sb[:, p:p + 1], None,
                        op0=MULT,
                    )
                nc.sync.dma_start(out_d[p * 128:(p + 1) * 128, :], vo_sb[:])

        finalize(0, NPAIR - 1)
        finalize(NPAIR - 1, NPAIR)

    nc.compile()
    return nc


def _get_program():
    if "nc" not in _CACHE:
        _CACHE["nc"] = _build_program()
    return _CACHE["nc"]


def _make_consts():
    eye = np.eye(K, dtype=np.float16)
    ones2c = np.zeros((128, 2), dtype=np.float32)
    ones2c[0:K, 0] = 1.0
    ones2c[K:128, 1] = 1.0
    sel2 = np.ascontiguousarray(ones2c.T)
    return {
        "eye128": np.eye(128, dtype=np.float16),
        "ones128": np.ones((128, 1), dtype=np.float16),
        "ones2c": ones2c,
        "sel2": sel2,
        "one1": np.ones((1, 1), dtype=np.float16),
    }


def kernel(x, W_assign, b_assign, C):
    global LAST_EXEC_NS, LAST_RESULTS

    x = np.asarray(x, dtype=np.float32).reshape(64, HW, D)
    W_assign = np.asarray(W_assign, dtype=np.float32)
    b_assign = np.asarray(b_assign, dtype=np.float32)
    C = np.asarray(C, dtype=np.float32)

    W16 = np.ascontiguousarray(
        W_assign.astype(np.float16).reshape(DC, 128, K).transpose(1, 0, 2)
    ).reshape(128, DC * K)
    bcol2 = np.concatenate([b_assign, b_assign]).reshape(128, 1).astype(np.float32)
    CT16 = np.ascontiguousarray(C.T).astype(np.float16)
    CT2 = np.vstack([CT16, CT16])
    consts = _make_consts()

    in_maps = []
    for c in range(N_CORES):
        xs = x[c * NSAMP:(c + 1) * NSAMP]
        xn16 = np.ascontiguousarray(
            xs.reshape(NPAIR, 2, HC, 128, D).transpose(0, 3, 1, 2, 4)
        ).reshape(NPAIR, 128, 2 * HC * D).astype(np.float16)
        xT = xs.transpose(0, 2, 1).reshape(NSAMP, DC, 128, HW)
        xT16 = np.ascontiguousarray(
            xT.transpose(0, 2, 1, 3)
        ).reshape(NSAMP, 128, DC * HW).astype(np.float16)
        in_maps.append({
            "xT": xT16, "xn": xn16, "Wt": W16, "bcol2": bcol2, "CT2": CT2,
            **consts,
        })

    nc = _get_program()
    trace = bool(int(os.environ.get("KERNEL_TRACE", "0")))
    res = run_bass_kernel_spmd(
        nc, in_maps, core_ids=list(range(N_CORES)), trace=trace,
    )
    LAST_RESULTS = res
    LAST_EXEC_NS = res.exec_time_ns

    out = np.empty((64, D * K), dtype=np.float32)
    for c in range(N_CORES):
        vT = res.results[c]["out"].reshape(NSAMP, K, D)
        out[c * NSAMP:(c + 1) * NSAMP] = (
            vT.transpose(0, 2, 1).reshape(NSAMP, D * K)
        )
    return out
